# revision 1
# baseline (speedup 1.0000x reference)
"""Trainium2 Bass kernel for nn_CrossAttention (masked+distance-modulated cross attention).

Sharding: 8 cores = batch(2) x head-groups(4). Each core computes 4 of 16 heads
for one batch element, producing a partial output projection; partials are
summed on host (Wp is row-sharded by head).

Per-core dataflow (activations feature-major [C, T]; DRAM inputs pre-tiled on
host so every DMA is a contiguous slab):
  K^T = Wk^T x_r^T, Q^T = Wq^T x_q^T   [256, 2048] f32r; x streamed in ck-pair
      slices; bias folded into the psum->sbuf copy (ACT Identity + bias AP).
  V   = sum_i y_i^T' Wv_i + bv          [2048, 256] f16, produced per s-chunk
      just-in-time inside the first attention pass.
  per (t1-range 512, head, s-chunk 128):
    S    = K^T(chunk)^T Q^T(range)      [128, 512] f32 psum
    E    = exp(S)                       ACT -> f16
    P1   = E * (mask*dmod)              DVE f16 (dmod = exp(-(d/gamma)^2))
    P2   = E * (mask*0.25)              DVE/GPSIMD split, fp8e4 out
    O[h] += V[chunk,h]^T P1             f16 matmul, psum rows 64*(h%2)
    den[h] += ones8^T P2                fp8 DoubleRow matmul, one pass per
      s-chunk PAIR, output = den replicated across all 128 psum rows
  O[h] /= den[h]: DVE reciprocal (psum->f16) + DVE mul -> oT f16
  Z = oT^T (Wp*0.25) -> psum -> f16 -> DMA out   (0.25 compensates the fp8
      mask scaling; host adds bp and sums the 4 head-group partials)
"""

import sys

sys.path.insert(0, "/opt/trn_rl_repo")

import numpy as np

import concourse.bass as bass
import concourse.mybir as mybir
import concourse.tile as tile
from concourse import bacc
from concourse.bass import ts
from concourse.bass_utils import run_bass_kernel_spmd

F32 = mybir.dt.float32
F32R = mybir.dt.float32r
F16 = mybir.dt.float16
F8E4 = mybir.dt.float8e4
Exp = mybir.ActivationFunctionType.Exp
Ident = mybir.ActivationFunctionType.Identity
DR = mybir.MatmulPerfMode.DoubleRow

# problem dims (hardcoded per contract)
B, T1, T2, C, NH, NI = 2, 2048, 2048, 1024, 16, 3
GAMMA = 0.5
NCORES = 8
HG = 4            # head groups (cores per batch)
HPG = NH // HG    # heads per group = 4
HD = C // NH      # 64
W = HPG * HD      # local width = 256

CK_N = C // 128   # 8 contraction chunks
JK_N = W // 128   # 2 output-partition chunks
TR_N = T1 // 512  # 4 t1 ranges
SC_N = T2 // 128  # 16 s chunks
TC_N = T1 // 128  # 16 t chunks for Z
ER_N = C // 512   # 2 output column ranges for Z

# fraction of P2 masks muls run on GPSIMD (Pool): (sc*2+i) % 16 < POOL_P2
POOL_P2 = 9


def build_core_program(dump=False):
    nc = bacc.Bacc(None, target_bir_lowering=False, debug=False)

    # pre-tiled inputs (host produces these layouts; all DMAs contiguous)
    xq = nc.declare_dram_parameter("xqT", [2, 4, 128, 2, 1024], F16, isOutput=False)
    xr = nc.declare_dram_parameter("xrT", [2, 4, 128, 2, 1024], F16, isOutput=False)
    yt = nc.declare_dram_parameter("yT", [SC_N, 128, NI, CK_N, 128], F16, isOutput=False)
    ffT = nc.declare_dram_parameter("ffT", [TR_N, 128, SC_N, 512], F16, isOutput=False)
    mfT = nc.declare_dram_parameter("mfT", [TR_N, 128, SC_N, 512], F8E4, isOutput=False)
    wq = nc.declare_dram_parameter("wq", [128, CK_N, JK_N, 128], F16, isOutput=False)
    wk = nc.declare_dram_parameter("wk", [128, CK_N, JK_N, 128], F16, isOutput=False)
    wv = nc.declare_dram_parameter("wv", [128, NI, CK_N, W], F16, isOutput=False)
    wp = nc.declare_dram_parameter("wp", [128, JK_N, C], F16, isOutput=False)
    bq2 = nc.declare_dram_parameter("bq2", [128, JK_N], F32, isOutput=False)
    bk2 = nc.declare_dram_parameter("bk2", [128, JK_N], F32, isOutput=False)
    bvs = nc.declare_dram_parameter("bvs", [1, W], F16, isOutput=False)
    ones1 = nc.declare_dram_parameter("ones1", [1, 128], F16, isOutput=False)
    ones8 = nc.declare_dram_parameter("ones8", [128, 2, 128], F8E4, isOutput=False)
    zp = nc.declare_dram_parameter("zpart", [TC_N, ER_N, 128, 512], F16, isOutput=True)

    with tile.TileContext(nc) as tc_:
        with tc_.tile_pool(name="persist", bufs=1) as pers:
            wk_sb = pers.tile([128, CK_N, JK_N, 128], F16, tag="wk")
            bk2_sb = pers.tile([128, JK_N], F32, tag="bk2")
            wq_sb = pers.tile([128, CK_N, JK_N, 128], F16, tag="wq")
            bq2_sb = pers.tile([128, JK_N], F32, tag="bq2")
            wv_sb = pers.tile([128, NI, CK_N, W], F16, tag="wv")
            wp_sb = pers.tile([128, JK_N, C], F16, tag="wp")
            bvs_sb = pers.tile([1, W], F16, tag="bvs")
            ones1_sb = pers.tile([1, 128], F16, tag="ones1")
            ones8_sb = pers.tile([128, 2, 128], F8E4, tag="ones8")

            # DMA queue is FIFO: order = dependency order of the program
            nc.sync.dma_start(wk_sb[:], wk[:])
            nc.sync.dma_start(bk2_sb[:], bk2[:])

            qT = pers.tile([128, JK_N, T1], F16, tag="qT")
            kT = pers.tile([128, JK_N, T2], F16, tag="kT")
            v_sb = pers.tile([128, SC_N, W], F16, tag="v")
            oT = pers.tile([128, JK_N, T1], F16, tag="oT")

            # ---- phase A: K^T then Q^T projections, x streamed per ck-pair ----
            with (
                tc_.tile_pool(name="pa", bufs=4, space="PSUM") as pa,
                tc_.tile_pool(name="xpool", bufs=3) as xpool,
            ):
                first = True
                for src, wsb, bias2, dst in (
                    (xr, wk_sb, bk2_sb, kT),
                    (xq, wq_sb, bq2_sb, qT),
                ):
                    for g in range(2):
                        pss = [pa.tile([128, 512], F32, tag="pa", name=f"pa{j}")
                               for j in range(4)]
                        for ckp in range(4):
                            x_sb = xpool.tile([128, 2, 1024], F16, tag="x")
                            nc.sync.dma_start(x_sb[:], src[g, ckp])
                            if first and ckp == 0:
                                # weights for the NEXT phases, after the first
                                # x slice so the K pipeline starts early
                                nc.sync.dma_start(wq_sb[:], wq[:])
                                nc.sync.dma_start(bq2_sb[:], bq2[:])
                                first = False
                            for cklo in range(2):
                                for jk in range(JK_N):
                                    for trl in range(2):
                                        nc.tensor.matmul(
                                            pss[jk * 2 + trl][:],
                                            wsb[:, ckp * 2 + cklo, jk, :],
                                            x_sb[:, cklo, ts(trl, 512)],
                                            start=(ckp == 0 and cklo == 0),
                                            stop=(ckp == 3 and cklo == 1),
                                        )
                        for jk in range(JK_N):
                            for trl in range(2):
                                nc.scalar.activation(
                                    dst[:, jk, ts(g * 2 + trl, 512)],
                                    pss[jk * 2 + trl][:],
                                    Ident,
                                    bias=bias2[:, jk:jk + 1],
                                    scale=1.0,
                                )

            # remaining weights + first mask slabs + y chunks: FIFO order
            nc.sync.dma_start(ones8_sb[:], ones8[:])
            nc.sync.dma_start(ones1_sb[:], ones1[:])
            nc.sync.dma_start(bvs_sb[:], bvs[:])
            nc.sync.dma_start(wv_sb[:], wv[:])

            mpool_cm = tc_.tile_pool(name="mpool", bufs=2)
            mpool = mpool_cm.__enter__()
            mslabs = {}

            def load_mslab(tr):
                ff_t = mpool.tile([128, SC_N, 512], F16, tag="ff", name=f"ff{tr}")
                mf_t = mpool.tile([128, SC_N, 512], F8E4, tag="mf", name=f"mf{tr}")
                nc.sync.dma_start(ff_t[:], ffT[tr])
                nc.sync.dma_start(mf_t[:], mfT[tr])
                mslabs[tr] = (ff_t, mf_t)

            load_mslab(0)

            # ---- phase B: attention; V fused into (tr0, hp0); Z interleaved ----
            with (
                tc_.tile_pool(name="spool", bufs=4, space="PSUM") as spool,
                tc_.tile_pool(name="po", bufs=2, space="PSUM") as popool,
                tc_.tile_pool(name="den", bufs=1, space="PSUM") as denpool,
                tc_.tile_pool(name="misc", bufs=1, space="PSUM") as misc,
                tc_.tile_pool(name="ypool", bufs=3) as ypool,
                tc_.tile_pool(name="epool", bufs=8) as epool,
                tc_.tile_pool(name="p1p", bufs=4) as p1p,
                tc_.tile_pool(name="p2p", bufs=33) as p2p,
                tc_.tile_pool(name="rbp", bufs=3) as rbp,
                tc_.tile_pool(name="zout", bufs=3) as zout,
            ):
                def emit_z(ztr):
                    for tcl in range(4):
                        tcc = ztr * 4 + tcl
                        for er in range(ER_N):
                            psz = misc.tile([128, 512], F32, tag="mz", name="psz")
                            for jk in range(JK_N):
                                nc.tensor.matmul(
                                    psz[:],
                                    oT[:, jk, ts(tcc, 128)],
                                    wp_sb[:, jk, ts(er, 512)],
                                    start=(jk == 0),
                                    stop=(jk == JK_N - 1),
                                )
                            z_t = zout.tile([128, 512], F16, tag="zt")
                            nc.scalar.copy(z_t[:], psz[:])
                            nc.sync.dma_start(zp[tcc, er], z_t[:])

                def flush_pending(pend):
                    # deferred denominator + normalization for the PREVIOUS
                    # head pair: every input (p2 tiles, po) is long since
                    # ready, so the PE never stalls on the slow P2 muls
                    ppo, pp2qs, ptr, php = pend
                    dens = [
                        denpool.tile([128, 512], F32, tag="den", name=f"den{i}")
                        for i in range(2)
                    ]
                    for scp in range(SC_N // 2):
                        for i in range(2):
                            nc.tensor.matmul(
                                dens[i][:],
                                ones8_sb[:],
                                pp2qs[i][scp][:],
                                start=(scp == 0),
                                stop=(scp == SC_N // 2 - 1),
                                perf_mode=DR,
                                tile_position=(0, 0),
                            )
                    for i in range(2):
                        off = 64 * i
                        rb = rbp.tile([128, 512], F16, tag="rb", name=f"rb{i}")
                        with nc.allow_low_precision(reason="softmax recip f16"):
                            nc.vector.reciprocal(rb[:], dens[i][:])
                        nc.vector.tensor_mul(
                            oT[off:off + 64, php, ts(ptr, 512)],
                            ppo[off:off + 64, :],
                            rb[off:off + 64, :],
                        )

                pend = None
                for tr in range(TR_N):
                    if tr not in mslabs:
                        load_mslab(tr)
                    ff_t, mf_t = mslabs.pop(tr)
                    if tr + 1 < TR_N and tr + 1 not in mslabs:
                        load_mslab(tr + 1)
                    if tr == 1:
                        nc.sync.dma_start(wp_sb[:], wp[:])
                    for hp in range(2):
                        po = popool.tile([128, 512], F32, tag="po")
                        p2qs = [[], []]
                        p2q = [None, None]
                        for sc in range(SC_N):
                            if sc == 1 and pend is not None:
                                flush_pending(pend)
                                pend = None
                            if tr == 0 and hp == 0:
                                # produce V[sc] just in time for the AV below
                                ysc = ypool.tile([128, NI, CK_N, 128], F16, tag="y")
                                nc.sync.dma_start(ysc[:], yt[sc])
                                pvt = misc.tile([128, 512], F32, tag="mz", name="pvt")
                                for ii in range(NI):
                                    for ck in range(CK_N):
                                        nc.tensor.matmul(
                                            pvt[:, 0:W],
                                            ysc[:, ii, ck, :],
                                            wv_sb[:, ii, ck, :],
                                            start=(ii == 0 and ck == 0),
                                            stop=False,
                                        )
                                nc.tensor.matmul(
                                    pvt[:, 0:W],
                                    ones1_sb[0:1, :],
                                    bvs_sb[0:1, :],
                                    start=False,
                                    stop=True,
                                )
                                nc.scalar.copy(v_sb[:, sc, :], pvt[:, 0:W])
                            if tr > 0 and hp == 0 and sc == 6:
                                emit_z(tr - 1)
                            for i in range(2):
                                h = 2 * hp + i
                                off = 64 * i
                                s_ps = spool.tile([128, 512], F32, tag="s", name=f"s{i}")
                                nc.tensor.matmul(
                                    s_ps[:],
                                    kT[off:off + 64, hp, ts(sc, 128)],
                                    qT[off:off + 64, hp, ts(tr, 512)],
                                    start=True,
                                    stop=True,
                                    tile_position=(off, 0),
                                )
                                e_t = epool.tile([128, 512], F16, tag="e", name=f"e{i}")
                                nc.scalar.activation(e_t[:], s_ps[:], Exp)
                                p1 = p1p.tile([128, 512], F16, tag="p1", name=f"p1_{i}")
                                nc.vector.tensor_mul(p1[:], e_t[:], ff_t[:, sc, :])
                                if sc % 2 == 0:
                                    p2q[i] = p2p.tile(
                                        [128, 2, 512], F8E4, tag="p2", name=f"p2_{i}"
                                    )
                                    p2qs[i].append(p2q[i])
                                if (sc * 2 + i) % 16 < POOL_P2:
                                    nc.gpsimd.tensor_mul(
                                        p2q[i][:, sc % 2, :], e_t[:], mf_t[:, sc, :]
                                    )
                                else:
                                    nc.vector.tensor_mul(
                                        p2q[i][:, sc % 2, :], e_t[:], mf_t[:, sc, :]
                                    )
                                nc.tensor.matmul(
                                    po[off:off + 64, :],
                                    v_sb[:, sc, ts(h, 64)],
                                    p1[:],
                                    start=(sc == 0),
                                    stop=(sc == SC_N - 1),
                                    tile_position=(0, off),
                                )
                        pend = (po, p2qs, tr, hp)
                flush_pending(pend)
                emit_z(TR_N - 1)

            mpool_cm.__exit__(None, None, None)

    nc.compile()
    return nc


_NC = None


def _get_nc():
    global _NC
    if _NC is None:
        _NC = build_core_program()
    return _NC


def make_in_maps(inputs):
    import ml_dtypes

    F8 = ml_dtypes.float8_e4m3fn
    x_q = np.asarray(inputs["x_q"], np.float32)
    x_r = np.asarray(inputs["x_r"], np.float32)
    y = np.asarray(inputs["y"], np.float32)
    mask = np.asarray(inputs["mask"])
    dist = np.asarray(inputs["dist"], np.float32)
    Wq, bq, Wk, bk, Wv, bv, Wp, bp = (
        np.asarray(inputs[k], np.float32)
        for k in ("Wq", "bq", "Wk", "bk", "Wv", "bv", "Wp", "bp")
    )

    s = np.float32(1.0 / np.sqrt(HD))

    per_batch = []
    for b in range(B):
        maskf = (mask[b, 0] != 0).astype(np.float32)  # [T1, T2]
        dmod = np.exp(-np.square(dist[b, 0] / GAMMA)).astype(np.float32)

        # [s, t] tiled as [tr, 128, sc, 512]
        def tile_st(a, dt):
            return np.ascontiguousarray(
                a.T.reshape(SC_N, 128, TR_N, 512).transpose(2, 1, 0, 3)
            ).astype(dt)

        ffT_ = tile_st(maskf * dmod, np.float16)
        mfT_ = tile_st(maskf * 0.25, F8)

        # x^T [c, t] -> [g2, ckp4, 128, cklo2, 1024]
        def tile_x(a):
            aT = a.T.reshape(4, 2, 128, 2, 1024)  # [ckp, cklo, p, g, t']
            return np.ascontiguousarray(aT.transpose(3, 0, 2, 1, 4)).astype(np.float16)

        xqT_ = tile_x(x_q[b])
        xrT_ = tile_x(x_r[b])
        # y[:, b] [NI, T2, C] -> [sc, 128p(c), NI, ck, 128(s)]
        yb = y[:, b].reshape(NI, SC_N, 128, CK_N, 128)  # i, sc, sl, ck, p
        yT_ = np.ascontiguousarray(yb.transpose(1, 4, 0, 3, 2)).astype(np.float16)
        per_batch.append((xqT_, xrT_, yT_, ffT_, mfT_))

    in_maps = []
    for core in range(NCORES):
        b, hg = divmod(core, HG)
        sl = slice(hg * W, (hg + 1) * W)
        xqT_, xrT_, yT_, ffT_, mfT_ = per_batch[b]
        wq_ = (Wq[:, sl] * s).reshape(CK_N, 128, JK_N, 128).transpose(1, 0, 2, 3)
        wk_ = Wk[:, sl].reshape(CK_N, 128, JK_N, 128).transpose(1, 0, 2, 3)
        wv_ = Wv[:, :, sl].reshape(NI, CK_N, 128, W).transpose(2, 0, 1, 3)
        # 0.25 compensates the fp8 mask scaling folded into the denominator
        wp_ = (Wp[sl, :] * 0.25).reshape(JK_N, 128, C).transpose(1, 0, 2)
        in_maps.append(
            {
                "xqT": xqT_,
                "xrT": xrT_,
                "yT": yT_,
                "ffT": ffT_,
                "mfT": mfT_,
                "wq": np.ascontiguousarray(wq_).astype(np.float16),
                "wk": np.ascontiguousarray(wk_).astype(np.float16),
                "wv": np.ascontiguousarray(wv_).astype(np.float16),
                "wp": np.ascontiguousarray(wp_).astype(np.float16),
                "bq2": np.ascontiguousarray(
                    (bq[sl] * s).reshape(JK_N, 128).T
                ).astype(np.float32),
                "bk2": np.ascontiguousarray(
                    bk[sl].reshape(JK_N, 128).T
                ).astype(np.float32),
                "bvs": bv.sum(0)[sl].reshape(1, W).astype(np.float16),
                "ones1": np.ones((1, 128), np.float16),
                "ones8": np.ones((128, 2, 128), F8),
            }
        )
    return in_maps


def kernel(x_q, x_r, y, mask, dist, Wq, bq, Wk, bk, Wv, bv, Wp, bp):
    inputs = dict(
        x_q=x_q, x_r=x_r, y=y, mask=mask, dist=dist,
        Wq=Wq, bq=bq, Wk=Wk, bk=bk, Wv=Wv, bv=bv, Wp=Wp, bp=bp,
    )
    in_maps = make_in_maps(inputs)
    nc = _get_nc()
    last = None
    for _ in range(3):
        try:
            res = run_bass_kernel_spmd(nc, in_maps, list(range(NCORES)))
            break
        except Exception as e:  # transient NRT device errors: retry
            last = e
    else:
        raise last

    out = np.zeros((B, T1, C), np.float32)
    for core in range(NCORES):
        b = core // HG
        z = res.results[core]["zpart"].astype(np.float32)  # [tc, er, 128, 512]
        out[b] += z.transpose(0, 2, 1, 3).reshape(T1, C)
    out += np.asarray(bp, np.float32)[None, None, :]
    return out

